# revision 1
# baseline (speedup 1.0000x reference)
"""Causal self-attention (B=4, T=2048, C=1024, H=16) on 8 Trainium2 cores.

Sharding: 2-way tensor parallel over head groups (8 heads each) x 4-way data
parallel over batch. Each core computes, for its (batch, head-group):
  - Q/K projection in transposed layout (Q^T, K^T = W^T @ x^T), bf16
  - V projection in natural [t, d] layout, bf16, with a ones-column appended
    per head so the PV matmul also produces the softmax denominator
  - causal attention in S^T = K Q^T orientation: exp (no max subtraction --
    logits are bounded ~O(3) for this problem scale), causal mask on diagonal
    128x128 sub-blocks, PV matmul accumulating U^T = [V|1]^T P^T
  - normalization y^T = U^T[:64] * (1/denom) broadcast via K=1 outer product
  - partial c_proj: part = y_local @ W_proj[rows of local heads]
Host sums the two head-group partials per batch and adds b_proj.

Head pairs are packed onto the 128x128 PE array (partitions 0-63 / 64-127)
so the K=64 S^T matmuls run concurrently in distinct row groups, and both
heads' scores share one [128, 2, 512] PSUM tile so a single ACTIVATE(Exp)
covers the pair (halves the per-instruction overhead on the scalar engine).
"""

import sys

sys.path.insert(0, "/opt/trn_rl_repo")

import numpy as np
import ml_dtypes

import concourse.bass as bass
import concourse.tile as tile
from concourse import mybir, bacc
from concourse import bass_utils
from concourse.bass import ts

# bass_utils imports antenv.axon_hooks when BASS_TRACE is set; the agent
# image's antenv may lack that module, so provide a no-op registry rather
# than crashing (tracing then degrades gracefully).
try:
    import antenv.axon_hooks  # noqa: F401
except ImportError:
    import types as _types
    import antenv as _antenv

    _ah = _types.ModuleType("antenv.axon_hooks")
    _ah._hook = None
    _ah.set_axon_ntff_profile_hook = lambda h, _m=_ah: setattr(_m, "_hook", h)
    _ah.get_axon_ntff_profile_hook = lambda _m=_ah: _m._hook
    sys.modules["antenv.axon_hooks"] = _ah
    _antenv.axon_hooks = _ah

BF16 = mybir.dt.bfloat16
F32 = mybir.dt.float32

B, T, C = 4, 2048, 1024
H, D = 16, 64
NG = 2               # head groups (tensor parallel)
HL = H // NG         # 8 local heads
PAIRS = HL // 2      # 4 head pairs (row/partition packing)
KC = C // 128        # 8 contraction tiles for projections
NT = T // 128        # 16 t tiles
NQ = T // 512        # 4 tq chunks
FT = (HL * D) // 128  # 4 feature tiles for c_proj contraction
N_CORES = 8

_CACHE = {}


def _build():
    nc = bacc.Bacc("TRN2", target_bir_lowering=False, debug=False,
                   num_devices=N_CORES)
    xT = nc.dram_tensor("xT", [C, T], BF16, kind="ExternalInput")
    W1 = nc.dram_tensor("W1", [C, 2 * HL * D], BF16, kind="ExternalInput")
    Wv = nc.dram_tensor("Wv", [C, HL * D], BF16, kind="ExternalInput")
    W2 = nc.dram_tensor("W2", [HL * D, C], BF16, kind="ExternalInput")
    bqk = nc.dram_tensor("bqk", [8, 128, 1], F32, kind="ExternalInput")
    bv = nc.dram_tensor("bv", [1, HL * D], F32, kind="ExternalInput")
    part = nc.dram_tensor("part", [T, C], F32, kind="ExternalOutput")

    EXP = mybir.ActivationFunctionType.Exp

    with tile.TileContext(nc) as tc:
        with (
            tc.tile_pool(name="const", bufs=1) as constp,
            tc.tile_pool(name="xw", bufs=1) as xw,
            tc.tile_pool(name="qkv", bufs=1) as qkv,
            tc.tile_pool(name="ytp", bufs=1) as ytp,
            tc.tile_pool(name="pt", bufs=18) as ptp,
            tc.tile_pool(name="rc", bufs=3) as rcp,
            tc.tile_pool(name="bco", bufs=3) as bcop,
            tc.tile_pool(name="outp", bufs=3) as outp,
            tc.tile_pool(name="psA", bufs=2, space="PSUM") as psA,
            tc.tile_pool(name="psS", bufs=2, space="PSUM") as psS,
            tc.tile_pool(name="psU", bufs=2, space="PSUM") as psU,
        ):
            # ---- constants / weights ----
            ones64 = constp.tile([1, 64], F32, tag="ones64")
            nc.vector.memset(ones64[:], 1.0)
            ones128 = constp.tile([1, 128], F32, tag="ones128")
            nc.vector.memset(ones128[:], 1.0)

            # weights on the gpsimd (SWDGE) rings, activations on sync
            # (HWDGE) so the transfers overlap; pair-0-needed data first
            W1_sb = xw.tile([128, KC, 2 * HL * D], BF16, tag="W1")
            W1r = W1.rearrange("(k p) m -> p k m", p=128)
            xT_sb = xw.tile([128, KC, T], BF16, tag="xT")
            xTr = xT.rearrange("(k p) t -> p k t", p=128)
            for kk in range(4):
                nc.sync.dma_start(W1_sb[:, 2 * kk:2 * kk + 2, :],
                                  W1r[:, 2 * kk:2 * kk + 2, :])
                nc.sync.dma_start(xT_sb[:, 2 * kk:2 * kk + 2, ts(0, 512)],
                                  xTr[:, 2 * kk:2 * kk + 2, ts(0, 512)])
            Wv_sb = xw.tile([128, KC, HL * D], BF16, tag="Wv")
            nc.sync.dma_start(Wv_sb[:], Wv.rearrange("(k p) m -> p k m", p=128))
            for q in range(1, NQ):
                nc.sync.dma_start(xT_sb[:, :, ts(q, 512)],
                                  xTr[:, :, ts(q, 512)])
            W2_sb = xw.tile([128, FT, C], BF16, tag="W2")
            bqk_sb = constp.tile([128, 8, 1], F32, tag="bqk")
            nc.sync.dma_start(bqk_sb[:], bqk.rearrange("j p o -> p j o"))
            bv_sb = constp.tile([1, HL * D], F32, tag="bv")
            nc.sync.dma_start(bv_sb[:], bv[:])

            # bv broadcast to all 128 t-rows: [128, 512] f32
            bvb_ps = psU.tile([128, HL * D], F32, tag="u")
            nc.tensor.matmul(bvb_ps[:], ones128[:], bv_sb[:], start=True, stop=True)
            bvb = constp.tile([128, HL * D], F32, tag="bvb")
            nc.vector.tensor_copy(bvb[:], bvb_ps[:])

            V_sb = qkv.tile([128, NT, HL * 65], BF16, tag="V")
            nc.gpsimd.memset(V_sb[:], 1.0)
            QT_sb = qkv.tile([128, PAIRS, T], BF16, tag="QT")
            KT_sb = qkv.tile([128, PAIRS, T], BF16, tag="KT")
            yT_sb = ytp.tile([128, PAIRS, T], BF16, tag="yT")

            def emit_v_group(i):
                # V projection t-tile i: V[t, d] (+bias), ones col per head
                acc = psA.tile([128, 512], F32, tag="acc")
                for k in range(KC):
                    nc.tensor.matmul(
                        acc[:], xT_sb[:, k, ts(i, 128)], Wv_sb[:, k, :],
                        start=(k == 0), stop=(k == KC - 1),
                    )
                # single strided add: psum [128,(8,64)] + bias -> V cols 0..63
                # of each 65-wide head block (col 64 stays the memset 1.0)
                vdst = V_sb[:, i, :].rearrange("p (h c) -> p h c", c=65)[:, :, 0:64]
                nc.vector.tensor_add(
                    vdst,
                    acc[:].rearrange("p (h c) -> p h c", c=64),
                    bvb[:].rearrange("p (h c) -> p h c", c=64))

            def emit_qk_group(pair, j, q):
                # Q/K projection: one [128, 512] output tile of Q^T or K^T
                acc = psA.tile([128, 512], F32, tag="acc")
                for k in range(KC):
                    nc.tensor.matmul(
                        acc[:], W1_sb[:, k, ts(j, 128)],
                        xT_sb[:, k, ts(q, 512)],
                        start=(k == 0), stop=(k == KC - 1),
                    )
                dst = QT_sb if j < 4 else KT_sb
                nc.vector.tensor_scalar_add(
                    dst[:, pair, ts(q, 512)], acc[:], bqk_sb[:, j, :])

            def emit_proj_group(i, n, tail=False):
                # c_proj partial: part[128i.., 512n..] = y_local @ W2_local
                acc = psA.tile([128, 512], F32, tag="acc")
                for k in range(FT):
                    nc.tensor.matmul(
                        acc[:], yT_sb[:, k, ts(i, 128)],
                        W2_sb[:, k, ts(n, 512)],
                        start=(k == 0), stop=(k == FT - 1),
                    )
                ot = outp.tile([128, 512], F32, tag="ot")
                if tail:
                    # ACT is exp-idle at the kernel tail; DVE is not
                    nc.scalar.copy(ot[:], acc[:])
                else:
                    nc.vector.tensor_copy(ot[:], acc[:])
                nc.sync.dma_start(part[ts(i, 128), ts(n, 512)], ot[:])

            def emit_attn_chunk(pair, q, filler):
                # attention for (pair, tq chunk q); pulls filler groups in
                # between to keep the PE busy while ACT runs the exps
                ntk = 4 * q + 4
                pts = []
                for i in range(ntk):
                    off = 128 * (i - 4 * q) if i >= 4 * q else 0
                    sS = psS.tile([128, 2, 512], F32, tag="s")
                    for a in range(2):
                        nc.tensor.matmul(
                            sS[:, a, off:512],
                            KT_sb[64 * a:64 * a + 64, pair, ts(i, 128)],
                            QT_sb[64 * a:64 * a + 64, pair,
                                  512 * q + off: 512 * (q + 1)],
                            start=True, stop=True,
                        )
                    pt = ptp.tile([128, 2, 512], BF16, tag="pt")
                    nc.scalar.activation(
                        pt[:, :, off:512], sS[:, :, off:512], EXP,
                        scale=0.125)
                    if i >= 4 * q:
                        # diagonal 128x128 sub-block: zero where tk > tq
                        for a in range(2):
                            nc.gpsimd.affine_select(
                                out=pt[:, a, off:off + 128],
                                in_=pt[:, a, off:off + 128],
                                compare_op=mybir.AluOpType.is_ge, fill=0.0,
                                base=0, pattern=[[1, 128]],
                                channel_multiplier=-1,
                            )
                    pts.append((pt, off))
                    if i % 3 == 2:
                        for f in filler.take():
                            f()
                for a in range(2):
                    h = 2 * pair + a
                    U = psU.tile([65, 512], F32, tag="u")
                    for i, (pt, off) in enumerate(pts):
                        nc.tensor.matmul(
                            U[:, off:512],
                            V_sb[:, i, 65 * h: 65 * h + 65],
                            pt[:, a, off:512],
                            start=(i == 0), stop=(i == ntk - 1),
                        )
                    # custom-DVE bitwise op requires SBUF input: stage
                    # the denominator row out of PSUM first
                    den = rcp.tile([1, 512], F32, tag="den")
                    nc.vector.tensor_copy(den[:], U[64:65, :])
                    recip = rcp.tile([1, 512], F32, tag="recip")
                    nc.vector.reciprocal_approx_fast(recip[:], den[:])
                    bcs = bcop.tile([64, 512], F32, tag="bcs")
                    nc.gpsimd.partition_broadcast(bcs[:], recip[:])
                    nc.vector.tensor_mul(
                        yT_sb[64 * a:64 * a + 64, pair, ts(q, 512)],
                        U[0:64, :], bcs[:])
                    for f in filler.take():
                        f()

            class Filler:
                """Doles out deferred PE work groups a couple at a time."""

                def __init__(self, groups, per_slot=1):
                    self.groups = list(groups)
                    self.per_slot = per_slot

                def take(self):
                    out, self.groups = (self.groups[:self.per_slot],
                                        self.groups[self.per_slot:])
                    return out

                def extend(self, groups):
                    self.groups.extend(groups)

                def drain(self):
                    for f in self.groups:
                        f()
                    self.groups = []

            # pair 0's Q/K projection runs up front; V tiles are emitted
            # just-in-time ahead of the PV groups that first need them
            for j in (0, 4):
                for q in range(NQ):
                    emit_qk_group(0, j, q)

            carry = []
            for pair in range(PAIRS):
                last = pair == PAIRS - 1
                groups = list(carry)
                carry = []
                if not last:
                    npair = pair + 1
                    groups += [
                        (lambda p_=npair, j=j, q=q: emit_qk_group(p_, j, q))
                        for q in range(2) for j in (npair, 4 + npair)
                    ]
                    # defer QK(npair) chunks 2-3 into pair npair's own
                    # early chunks so its PE never runs dry
                    carry = [
                        (lambda p_=npair, j=j, q=q: emit_qk_group(p_, j, q))
                        for q in range(2, NQ) for j in (npair, 4 + npair)
                    ]
                filler = Filler(groups, per_slot=3 if last else 1)
                if pair == 1:
                    nc.scalar.dma_start(
                        W2_sb[:], W2.rearrange("(k p) m -> p k m", p=128))
                for q in range(NQ):
                    if pair == 0:
                        for i in range(4 * q, 4 * q + 4):
                            emit_v_group(i)
                    emit_attn_chunk(pair, q, filler)
                    if last:
                        # yT chunk q is complete across all pairs: its
                        # c_proj tiles become filler for the next chunk
                        filler.extend([
                            (lambda i=i, n=n, t=(q == NQ - 1):
                             emit_proj_group(i, n, tail=t))
                            for i in range(4 * q, 4 * q + 4)
                            for n in range(2)
                        ])
                filler.drain()

    nc.compile()
    return nc


def _get_nc():
    if "nc" not in _CACHE:
        _CACHE["nc"] = _build()
    return _CACHE["nc"]


def _prep_in_maps(x, W_attn, b_attn, W_proj):
    bf = ml_dtypes.bfloat16
    in_maps = []
    gw = {}
    for g in range(NG):
        s = slice(512 * g, 512 * g + 512)
        W1l = np.concatenate(
            [W_attn[:, 0 * C:][:, s], W_attn[:, 1 * C:][:, s]], axis=1
        ).astype(bf)
        Wvl = W_attn[:, 2 * C:][:, s].astype(bf)
        W2l = np.ascontiguousarray(W_proj[s, :]).astype(bf)
        bqkl = np.concatenate(
            [b_attn[0 * C:][s], b_attn[1 * C:][s]]
        ).astype(np.float32).reshape(8, 128, 1)
        bvl = b_attn[2 * C:][s].astype(np.float32).reshape(1, 512)
        gw[g] = (W1l, Wvl, W2l, bqkl, bvl)
    for b in range(B):
        xTl = np.ascontiguousarray(x[b].T).astype(bf)
        for g in range(NG):
            W1l, Wvl, W2l, bqkl, bvl = gw[g]
            in_maps.append({"xT": xTl, "W1": W1l, "Wv": Wvl, "W2": W2l,
                            "bqk": bqkl, "bv": bvl})
    return in_maps


LAST_RESULTS = None


def kernel(x, W_attn, b_attn, W_proj, b_proj):
    global LAST_RESULTS
    nc = _get_nc()
    in_maps = _prep_in_maps(np.asarray(x, np.float32),
                            np.asarray(W_attn, np.float32),
                            np.asarray(b_attn, np.float32),
                            np.asarray(W_proj, np.float32))
    res = bass_utils.run_bass_kernel_spmd(nc, in_maps,
                                          core_ids=list(range(N_CORES)))
    LAST_RESULTS = res
    out = np.empty((B, T, C), np.float32)
    bp = np.asarray(b_proj, np.float32)
    for b in range(B):
        out[b] = res.results[2 * b]["part"] + res.results[2 * b + 1]["part"] + bp
    return out



# revision 3
# speedup vs baseline: 1.0481x; 1.0481x over previous
"""Causal self-attention (B=4, T=2048, C=1024, H=16) on 8 Trainium2 cores.

Sharding: 2-way tensor parallel over head groups (8 heads each) x 4-way data
parallel over batch. Each core computes, for its (batch, head-group):
  - Q/K projection in transposed layout (Q^T, K^T = W^T @ x^T), bf16
  - V projection in natural [t, d] layout, bf16, with a ones-column appended
    per head so the PV matmul also produces the softmax denominator
  - causal attention in S^T = K Q^T orientation: exp (no max subtraction --
    logits are bounded ~O(3) for this problem scale), causal mask on diagonal
    128x128 sub-blocks, PV matmul accumulating U^T = [V|1]^T P^T
  - normalization y^T = U^T[:64] * (1/denom) broadcast via K=1 outer product
  - partial c_proj: part = y_local @ W_proj[rows of local heads]
Host sums the two head-group partials per batch and adds b_proj.

Head pairs are packed onto the 128x128 PE array (partitions 0-63 / 64-127)
so the K=64 S^T matmuls run concurrently in distinct row groups, and both
heads' scores share one [128, 2, 512] PSUM tile so a single ACTIVATE(Exp)
covers the pair (halves the per-instruction overhead on the scalar engine).

Schedule is chunk-major (tq chunk outer, head-pair inner) so c_proj tiles of
chunk q run as PE filler during chunk q+1 instead of crowding the kernel
tail, and the startup emits only the two QK projection groups the first
attention chunk needs (the rest arrive as filler while attention runs).
W1 is pre-packed j-major on the host so each projection tile's weights
arrive in one contiguous-run DMA slice, ordered by first use.
"""

import sys

sys.path.insert(0, "/opt/trn_rl_repo")

import numpy as np
import ml_dtypes

import concourse.bass as bass
import concourse.tile as tile
from concourse import mybir, bacc
from concourse import bass_utils
from concourse.bass import ts

# bass_utils imports antenv.axon_hooks when BASS_TRACE is set; the agent
# image's antenv may lack that module, so provide a no-op registry rather
# than crashing (tracing then degrades gracefully).
try:
    import antenv.axon_hooks  # noqa: F401
except ImportError:
    import types as _types
    import antenv as _antenv

    _ah = _types.ModuleType("antenv.axon_hooks")
    _ah._hook = None
    _ah.set_axon_ntff_profile_hook = lambda h, _m=_ah: setattr(_m, "_hook", h)
    _ah.get_axon_ntff_profile_hook = lambda _m=_ah: _m._hook
    sys.modules["antenv.axon_hooks"] = _ah
    _antenv.axon_hooks = _ah

BF16 = mybir.dt.bfloat16
F32 = mybir.dt.float32

B, T, C = 4, 2048, 1024
H, D = 16, 64
NG = 2               # head groups (tensor parallel)
HL = H // NG         # 8 local heads
PAIRS = HL // 2      # 4 head pairs (row/partition packing)
KC = C // 128        # 8 contraction tiles for projections
NT = T // 128        # 16 t tiles
NQ = T // 512        # 4 tq chunks
FT = (HL * D) // 128  # 4 feature tiles for c_proj contraction
N_CORES = 8

_CACHE = {}


def _build():
    nc = bacc.Bacc("TRN2", target_bir_lowering=False, debug=False,
                   num_devices=N_CORES)
    xT = nc.dram_tensor("xT", [C, T], BF16, kind="ExternalInput")
    W1 = nc.dram_tensor("W1", [128, 8, KC, 128], BF16, kind="ExternalInput")
    Wv = nc.dram_tensor("Wv", [C, HL * D], BF16, kind="ExternalInput")
    W2 = nc.dram_tensor("W2", [HL * D, C], BF16, kind="ExternalInput")
    bqk = nc.dram_tensor("bqk", [8, 128, 1], F32, kind="ExternalInput")
    bv = nc.dram_tensor("bv", [1, HL * D], F32, kind="ExternalInput")
    part = nc.dram_tensor("part", [T, C], F32, kind="ExternalOutput")

    EXP = mybir.ActivationFunctionType.Exp

    with tile.TileContext(nc) as tc:
        with (
            tc.tile_pool(name="const", bufs=1) as constp,
            tc.tile_pool(name="xw", bufs=1) as xw,
            tc.tile_pool(name="qkv", bufs=1) as qkv,
            tc.tile_pool(name="ytp", bufs=1) as ytp,
            tc.tile_pool(name="pt", bufs=18) as ptp,
            tc.tile_pool(name="rc", bufs=3) as rcp,
            tc.tile_pool(name="bco", bufs=3) as bcop,
            tc.tile_pool(name="outp", bufs=3) as outp,
            tc.tile_pool(name="psA", bufs=2, space="PSUM") as psA,
            tc.tile_pool(name="psS", bufs=2, space="PSUM") as psS,
            tc.tile_pool(name="psU", bufs=2, space="PSUM") as psU,
        ):
            # ---- constants / weights ----
            ones64 = constp.tile([1, 64], F32, tag="ones64")
            nc.vector.memset(ones64[:], 1.0)
            ones128 = constp.tile([1, 128], F32, tag="ones128")
            nc.vector.memset(ones128[:], 1.0)

            # DMA schedule: sync (HWDGE) carries the startup-critical stream
            # in exact first-use order; W2 rides the scalar ring in parallel.
            bqk_sb = constp.tile([128, 8, 1], F32, tag="bqk")
            nc.sync.dma_start(bqk_sb[:], bqk.rearrange("j p o -> p j o"))
            bv_sb = constp.tile([1, HL * D], F32, tag="bv")
            nc.sync.dma_start(bv_sb[:], bv[:])

            xT_sb = xw.tile([128, KC, T], BF16, tag="xT")
            xTr = xT.rearrange("(k p) t -> p k t", p=128)
            nc.sync.dma_start(xT_sb[:, :, ts(0, 512)], xTr[:, :, ts(0, 512)])
            W1_sb = xw.tile([128, 8, KC, 128], BF16, tag="W1")
            for j in (0, 4):
                nc.sync.dma_start(W1_sb[:, j], W1[:, j])
            Wv_sb = xw.tile([128, KC, HL * D], BF16, tag="Wv")
            nc.sync.dma_start(Wv_sb[:], Wv.rearrange("(k p) m -> p k m", p=128))
            for j in (1, 5, 2, 6, 3, 7):
                nc.sync.dma_start(W1_sb[:, j], W1[:, j])
            for q in range(1, NQ):
                nc.sync.dma_start(xT_sb[:, :, ts(q, 512)],
                                  xTr[:, :, ts(q, 512)])
            W2_sb = xw.tile([128, FT, C], BF16, tag="W2")
            nc.scalar.dma_start(W2_sb[:], W2.rearrange("(k p) m -> p k m", p=128))

            # bv broadcast to all 128 t-rows: [128, 512] f32
            bvb_ps = psU.tile([128, HL * D], F32, tag="u")
            nc.tensor.matmul(bvb_ps[:], ones128[:], bv_sb[:], start=True, stop=True)
            bvb = constp.tile([128, HL * D], F32, tag="bvb")
            nc.vector.tensor_copy(bvb[:], bvb_ps[:])

            V_sb = qkv.tile([128, NT, HL, 65], BF16, tag="V")
            # only the per-head ones-column needs the memset; the V columns
            # are fully overwritten by the projection's bias-add below
            nc.gpsimd.memset(V_sb[:, :, :, 64:65], 1.0)
            QT_sb = qkv.tile([128, PAIRS, T], BF16, tag="QT")
            KT_sb = qkv.tile([128, PAIRS, T], BF16, tag="KT")
            yT_sb = ytp.tile([128, PAIRS, T], BF16, tag="yT")

            def emit_v_group(i):
                # V projection t-tile i: V[t, d] (+bias), ones col per head
                acc = psA.tile([128, 512], F32, tag="acc")
                for k in range(KC):
                    nc.tensor.matmul(
                        acc[:], xT_sb[:, k, ts(i, 128)], Wv_sb[:, k, :],
                        start=(k == 0), stop=(k == KC - 1),
                    )
                # single strided add: psum [128,(8,64)] + bias -> V cols 0..63
                # of each 65-wide head block (col 64 stays the memset 1.0)
                nc.vector.tensor_add(
                    V_sb[:, i, :, 0:64],
                    acc[:].rearrange("p (h c) -> p h c", c=64),
                    bvb[:].rearrange("p (h c) -> p h c", c=64))

            def emit_qk_group(pair, j, q):
                # Q/K projection: one [128, 512] output tile of Q^T or K^T
                acc = psA.tile([128, 512], F32, tag="acc")
                for k in range(KC):
                    nc.tensor.matmul(
                        acc[:], W1_sb[:, j, k, :],
                        xT_sb[:, k, ts(q, 512)],
                        start=(k == 0), stop=(k == KC - 1),
                    )
                dst = QT_sb if j < 4 else KT_sb
                nc.vector.tensor_scalar_add(
                    dst[:, pair, ts(q, 512)], acc[:], bqk_sb[:, j, :])

            def emit_proj_group(i, n, tail=False):
                # c_proj partial: part[128i.., 512n..] = y_local @ W2_local
                acc = psA.tile([128, 512], F32, tag="acc")
                for k in range(FT):
                    nc.tensor.matmul(
                        acc[:], yT_sb[:, k, ts(i, 128)],
                        W2_sb[:, k, ts(n, 512)],
                        start=(k == 0), stop=(k == FT - 1),
                    )
                ot = outp.tile([128, 512], F32, tag="ot")
                if tail:
                    # ACT is exp-idle at the kernel tail; DVE is not
                    nc.scalar.copy(ot[:], acc[:])
                else:
                    nc.vector.tensor_copy(ot[:], acc[:])
                nc.sync.dma_start(part[ts(i, 128), ts(n, 512)], ot[:])

            def emit_attn_chunk(pair, q, filler, every_tile=False):
                # attention for (pair, tq chunk q); pulls filler groups in
                # between to keep the PE busy while ACT runs the exps
                ntk = 4 * q + 4
                pts = []
                for i in range(ntk):
                    off = 128 * (i - 4 * q) if i >= 4 * q else 0
                    sS = psS.tile([128, 2, 512], F32, tag="s")
                    for a in range(2):
                        nc.tensor.matmul(
                            sS[:, a, off:512],
                            KT_sb[64 * a:64 * a + 64, pair, ts(i, 128)],
                            QT_sb[64 * a:64 * a + 64, pair,
                                  512 * q + off: 512 * (q + 1)],
                            start=True, stop=True,
                        )
                    pt = ptp.tile([128, 2, 512], BF16, tag="pt")
                    nc.scalar.activation(
                        pt[:, :, off:512], sS[:, :, off:512], EXP,
                        scale=0.125)
                    if i >= 4 * q:
                        # diagonal 128x128 sub-block: zero where tk > tq
                        for a in range(2):
                            nc.gpsimd.affine_select(
                                out=pt[:, a, off:off + 128],
                                in_=pt[:, a, off:off + 128],
                                compare_op=mybir.AluOpType.is_ge, fill=0.0,
                                base=0, pattern=[[1, 128]],
                                channel_multiplier=-1,
                            )
                    pts.append((pt, off))
                    if every_tile or i % 3 == 2:
                        for f in filler.take():
                            f()
                for a in range(2):
                    h = 2 * pair + a
                    U = psU.tile([65, 512], F32, tag="u")
                    for i, (pt, off) in enumerate(pts):
                        nc.tensor.matmul(
                            U[:, off:512],
                            V_sb[:, i, h, :],
                            pt[:, a, off:512],
                            start=(i == 0), stop=(i == ntk - 1),
                        )
                    # custom-DVE bitwise op requires SBUF input: stage
                    # the denominator row out of PSUM first
                    den = rcp.tile([1, 512], F32, tag="den")
                    nc.vector.tensor_copy(den[:], U[64:65, :])
                    recip = rcp.tile([1, 512], F32, tag="recip")
                    nc.vector.reciprocal_approx_fast(recip[:], den[:])
                    bcs = bcop.tile([64, 512], F32, tag="bcs")
                    nc.gpsimd.partition_broadcast(bcs[:], recip[:])
                    nc.vector.tensor_mul(
                        yT_sb[64 * a:64 * a + 64, pair, ts(q, 512)],
                        U[0:64, :], bcs[:])
                    for f in filler.take():
                        f()

            class Filler:
                """Doles out deferred, tagged PE work groups a few at a time.
                force() emits immediately any queued group a consumer is
                about to depend on (dataflow deps only see prior writes)."""

                def __init__(self, per_slot=1):
                    self.groups = []
                    self.per_slot = per_slot

                def take(self):
                    out, self.groups = (self.groups[:self.per_slot],
                                        self.groups[self.per_slot:])
                    return [f for _, f in out]

                def extend(self, tagged):
                    self.groups.extend(tagged)

                def force(self, pred):
                    keep = []
                    for tag, f in self.groups:
                        if pred(tag):
                            f()
                        else:
                            keep.append((tag, f))
                    self.groups = keep

                def drain(self):
                    for _, f in self.groups:
                        f()
                    self.groups = []

            # startup: only the two projection groups attn(pair0, q0) needs
            emit_qk_group(0, 0, 0)
            emit_qk_group(0, 4, 0)

            filler = Filler()
            for q in range(NQ):
                for pair in range(PAIRS):
                    if q == 0 and pair == 0:
                        # first chunk: V tiles 0-3 feed PV almost at once
                        for i in range(4):
                            emit_v_group(i)
                    # queue work for upcoming consumers, in need order
                    if pair < PAIRS - 1:
                        npair = pair + 1
                        filler.extend([
                            (("qk", npair, q),
                             lambda p_=npair, j_=j_, q_=q: emit_qk_group(p_, j_, q_))
                            for j_ in (npair, 4 + npair)
                        ])
                    elif q < NQ - 1:
                        filler.extend([
                            (("qk", 0, q + 1),
                             lambda j_=j_, q_=q + 1: emit_qk_group(0, j_, q_))
                            for j_ in (0, 4)
                        ])
                    if q < NQ - 1:
                        # V tiles for chunk q+1, spread over pairs 1-2
                        if pair == 1:
                            filler.extend([
                                (("v", i), lambda i=i: emit_v_group(i))
                                for i in (4 * q + 4, 4 * q + 5)
                            ])
                        elif pair == 2:
                            filler.extend([
                                (("v", i), lambda i=i: emit_v_group(i))
                                for i in (4 * q + 6, 4 * q + 7)
                            ])
                    # correctness: everything attn(pair, q) reads must be
                    # emitted before it (QK of this (pair, q); V tiles < ntk)
                    filler.force(lambda tag, p_=pair, q_=q: (
                        tag == ("qk", p_, q_)
                        or (tag[0] == "v" and tag[1] < 4 * q_ + 4)))
                    emit_attn_chunk(pair, q, filler,
                                    every_tile=(q <= 1))
                # yT chunk q complete across all pairs: its c_proj tiles
                # become filler for chunk q+1 (drained at the end for q=3)
                filler.extend([
                    (("proj", q),
                     lambda i=i, n=n, t=(q == NQ - 1):
                     emit_proj_group(i, n, tail=t))
                    for i in range(4 * q, 4 * q + 4)
                    for n in range(2)
                ])
            filler.drain()

    nc.compile()
    return nc


def _get_nc():
    if "nc" not in _CACHE:
        _CACHE["nc"] = _build()
    return _CACHE["nc"]


def _prep_in_maps(x, W_attn, b_attn, W_proj):
    bf = ml_dtypes.bfloat16
    in_maps = []
    gw = {}
    for g in range(NG):
        s = slice(512 * g, 512 * g + 512)
        W1l = np.concatenate(
            [W_attn[:, 0 * C:][:, s], W_attn[:, 1 * C:][:, s]], axis=1
        )
        # j-major pack: [p, j, k, c] so each j-tile is one contiguous DMA
        W1l = np.ascontiguousarray(
            W1l.reshape(KC, 128, 8, 128).transpose(1, 2, 0, 3)
        ).astype(bf)
        Wvl = W_attn[:, 2 * C:][:, s].astype(bf)
        W2l = np.ascontiguousarray(W_proj[s, :]).astype(bf)
        bqkl = np.concatenate(
            [b_attn[0 * C:][s], b_attn[1 * C:][s]]
        ).astype(np.float32).reshape(8, 128, 1)
        bvl = b_attn[2 * C:][s].astype(np.float32).reshape(1, 512)
        gw[g] = (W1l, Wvl, W2l, bqkl, bvl)
    for b in range(B):
        xTl = np.ascontiguousarray(x[b].T).astype(bf)
        for g in range(NG):
            W1l, Wvl, W2l, bqkl, bvl = gw[g]
            in_maps.append({"xT": xTl, "W1": W1l, "Wv": Wvl, "W2": W2l,
                            "bqk": bqkl, "bv": bvl})
    return in_maps


LAST_RESULTS = None


def kernel(x, W_attn, b_attn, W_proj, b_proj):
    global LAST_RESULTS
    nc = _get_nc()
    in_maps = _prep_in_maps(np.asarray(x, np.float32),
                            np.asarray(W_attn, np.float32),
                            np.asarray(b_attn, np.float32),
                            np.asarray(W_proj, np.float32))
    res = bass_utils.run_bass_kernel_spmd(nc, in_maps,
                                          core_ids=list(range(N_CORES)))
    LAST_RESULTS = res
    out = np.empty((B, T, C), np.float32)
    bp = np.asarray(b_proj, np.float32)
    for b in range(B):
        out[b] = res.results[2 * b]["part"] + res.results[2 * b + 1]["part"] + bp
    return out


# revision 9
# speedup vs baseline: 1.0632x; 1.0145x over previous
"""Causal self-attention (B=4, T=2048, C=1024, H=16) on 8 Trainium2 cores.

Sharding: 2-way tensor parallel over head groups (8 heads each) x 4-way data
parallel over batch. Each core computes, for its (batch, head-group):
  - Q/K projection in transposed layout (Q^T, K^T = W^T @ x^T), bf16
  - V projection in natural [t, d] layout, bf16, with a ones-column appended
    per head so the PV matmul also produces the softmax denominator
  - causal attention in S^T = K Q^T orientation: exp (no max subtraction --
    logits are bounded ~O(3) for this problem scale), causal mask on diagonal
    128x128 sub-blocks, PV matmul accumulating U^T = [V|1]^T P^T
  - normalization y^T = U^T[:64] * (1/denom) broadcast via K=1 outer product
  - partial c_proj: part = y_local @ W_proj[rows of local heads]
Host sums the two head-group partials per batch and adds b_proj.

Head pairs are packed onto the 128x128 PE array (partitions 0-63 / 64-127)
so the K=64 S^T matmuls run concurrently in distinct row groups, and both
heads' scores share one [128, 2, 512] PSUM tile so a single ACTIVATE(Exp)
covers the pair (halves the per-instruction overhead on the scalar engine).

Schedule is chunk-major (tq chunk outer, head-pair inner) so c_proj tiles of
chunk q run as PE filler during chunk q+1 instead of crowding the kernel
tail, and the startup emits only the two QK projection groups the first
attention chunk needs (the rest arrive as filler while attention runs).
W1 is pre-packed j-major on the host so each projection tile's weights
arrive in one contiguous-run DMA slice, ordered by first use.
"""

import sys

sys.path.insert(0, "/opt/trn_rl_repo")

import numpy as np
import ml_dtypes

import concourse.bass as bass
import concourse.tile as tile
from concourse import mybir, bacc
from concourse import bass_utils
from concourse.bass import ts

# bass_utils imports antenv.axon_hooks when BASS_TRACE is set; the agent
# image's antenv may lack that module, so provide a no-op registry rather
# than crashing (tracing then degrades gracefully).
try:
    import antenv.axon_hooks  # noqa: F401
except ImportError:
    import types as _types
    import antenv as _antenv

    _ah = _types.ModuleType("antenv.axon_hooks")
    _ah._hook = None
    _ah.set_axon_ntff_profile_hook = lambda h, _m=_ah: setattr(_m, "_hook", h)
    _ah.get_axon_ntff_profile_hook = lambda _m=_ah: _m._hook
    sys.modules["antenv.axon_hooks"] = _ah
    _antenv.axon_hooks = _ah

BF16 = mybir.dt.bfloat16
F32 = mybir.dt.float32
FP8 = mybir.dt.float8e4
DBLROW = mybir.MatmulPerfMode.DoubleRow
# Q/K projection runs in fp8e4 DoubleRow (2 contraction tiles per pass).
# W1 is pre-scaled by W1_SCALE on the host so its 0.02-std weights clear the
# e4m3 subnormal threshold; Q^T/K^T come out W1_SCALE x too large, which the
# exp's logit scale divides back out (bqk is pre-scaled to match).
W1_SCALE = 256.0

B, T, C = 4, 2048, 1024
H, D = 16, 64
NG = 2               # head groups (tensor parallel)
HL = H // NG         # 8 local heads
PAIRS = HL // 2      # 4 head pairs (row/partition packing)
KC = C // 128        # 8 contraction tiles for projections
NT = T // 128        # 16 t tiles
NQ = T // 512        # 4 tq chunks
FT = (HL * D) // 128  # 4 feature tiles for c_proj contraction
N_CORES = 8

_CACHE = {}


def _build():
    nc = bacc.Bacc("TRN2", target_bir_lowering=False, debug=False,
                   num_devices=N_CORES)
    xT = nc.dram_tensor("xT", [C, T], BF16, kind="ExternalInput")
    xT8 = nc.dram_tensor("xT8", [C, T], FP8, kind="ExternalInput")
    W1 = nc.dram_tensor("W1", [128, 8, KC, 128], FP8, kind="ExternalInput")
    Wv = nc.dram_tensor("Wv", [C, HL * D], BF16, kind="ExternalInput")
    W2 = nc.dram_tensor("W2", [HL * D, C], BF16, kind="ExternalInput")
    bqk = nc.dram_tensor("bqk", [8, 128, 1], F32, kind="ExternalInput")
    bv = nc.dram_tensor("bv", [1, HL * D], F32, kind="ExternalInput")
    part = nc.dram_tensor("part", [T, C], F32, kind="ExternalOutput")

    EXP = mybir.ActivationFunctionType.Exp

    with tile.TileContext(nc) as tc:
        with (
            tc.tile_pool(name="const", bufs=1) as constp,
            tc.tile_pool(name="xw", bufs=1) as xw,
            tc.tile_pool(name="qkv", bufs=1) as qkv,
            tc.tile_pool(name="ytp", bufs=1) as ytp,
            tc.tile_pool(name="pt", bufs=18) as ptp,
            tc.tile_pool(name="rc", bufs=3) as rcp,
            tc.tile_pool(name="bco", bufs=3) as bcop,
            tc.tile_pool(name="outp", bufs=3) as outp,
            tc.tile_pool(name="psA", bufs=2, space="PSUM") as psA,
            tc.tile_pool(name="psS", bufs=2, space="PSUM") as psS,
            tc.tile_pool(name="psU", bufs=2, space="PSUM") as psU,
        ):
            # ---- constants / weights ----
            ones64 = constp.tile([1, 64], F32, tag="ones64")
            nc.vector.memset(ones64[:], 1.0)
            ones128 = constp.tile([1, 128], F32, tag="ones128")
            nc.vector.memset(ones128[:], 1.0)

            # DMA schedule: sync (HWDGE) carries the startup-critical stream
            # in exact first-use order; W2 rides the scalar ring in parallel.
            bqk_sb = constp.tile([128, 8, 1], F32, tag="bqk")
            nc.sync.dma_start(bqk_sb[:], bqk.rearrange("j p o -> p j o"))
            bv_sb = constp.tile([1, HL * D], F32, tag="bv")
            nc.sync.dma_start(bv_sb[:], bv[:])

            xT8_sb = xw.tile([128, KC, T], FP8, tag="xT8")
            xT8r = xT8.rearrange("(k p) t -> p k t", p=128)
            nc.sync.dma_start(xT8_sb[:, :, ts(0, 512)], xT8r[:, :, ts(0, 512)])
            W1_sb = xw.tile([128, 8, KC, 128], FP8, tag="W1")
            for j in (0, 4):
                nc.sync.dma_start(W1_sb[:, j], W1[:, j])
            xT_sb = xw.tile([128, KC, T], BF16, tag="xT")
            xTr = xT.rearrange("(k p) t -> p k t", p=128)
            nc.sync.dma_start(xT_sb[:, :, ts(0, 512)], xTr[:, :, ts(0, 512)])
            Wv_sb = xw.tile([128, KC, HL * D], BF16, tag="Wv")
            nc.sync.dma_start(Wv_sb[:], Wv.rearrange("(k p) m -> p k m", p=128))
            for j in (1, 5, 2, 6, 3, 7):
                nc.sync.dma_start(W1_sb[:, j], W1[:, j])
            nc.sync.dma_start(xT8_sb[:, :, ts(1, 512)], xT8r[:, :, ts(1, 512)])
            for q in range(1, NQ):
                nc.sync.dma_start(xT_sb[:, :, ts(q, 512)],
                                  xTr[:, :, ts(q, 512)])
                if q < NQ - 1:
                    nc.sync.dma_start(xT8_sb[:, :, ts(q + 1, 512)],
                                      xT8r[:, :, ts(q + 1, 512)])
            W2_sb = xw.tile([128, FT, C], BF16, tag="W2")
            nc.scalar.dma_start(W2_sb[:], W2.rearrange("(k p) m -> p k m", p=128))

            # bv broadcast to all 128 t-rows: [128, 512] f32
            bvb_ps = psU.tile([128, HL * D], F32, tag="u")
            nc.tensor.matmul(bvb_ps[:], ones128[:], bv_sb[:], start=True, stop=True)
            bvb = constp.tile([128, HL * D], F32, tag="bvb")
            nc.vector.tensor_copy(bvb[:], bvb_ps[:])

            V_sb = qkv.tile([128, NT, HL, 65], BF16, tag="V")
            # only the per-head ones-column needs the memset; the V columns
            # are fully overwritten by the projection's bias-add below
            nc.gpsimd.memset(V_sb[:, :, :, 64:65], 1.0)
            QT_sb = qkv.tile([128, PAIRS, T], BF16, tag="QT")
            KT_sb = qkv.tile([128, PAIRS, T], BF16, tag="KT")
            yT_sb = ytp.tile([128, PAIRS, T], BF16, tag="yT")

            def emit_v_group(i):
                # V projection t-tile i: V[t, d] (+bias), ones col per head
                acc = psA.tile([128, 512], F32, tag="acc")
                for k in range(KC):
                    nc.tensor.matmul(
                        acc[:], xT_sb[:, k, ts(i, 128)], Wv_sb[:, k, :],
                        start=(k == 0), stop=(k == KC - 1),
                    )
                # single strided add: psum [128,(8,64)] + bias -> V cols 0..63
                # of each 65-wide head block (col 64 stays the memset 1.0)
                nc.vector.tensor_add(
                    V_sb[:, i, :, 0:64],
                    acc[:].rearrange("p (h c) -> p h c", c=64),
                    bvb[:].rearrange("p (h c) -> p h c", c=64))

            def emit_qk_group(pair, j, q):
                # Q/K projection: one [128, 512] output tile of Q^T or K^T,
                # fp8e4 DoubleRow -- each pass contracts 2 k-tiles (K=256)
                acc = psA.tile([128, 512], F32, tag="acc")
                for k2 in range(KC // 2):
                    nc.tensor.matmul(
                        acc[:], W1_sb[:, j, 2 * k2:2 * k2 + 2, :],
                        xT8_sb[:, 2 * k2:2 * k2 + 2, ts(q, 512)],
                        start=(k2 == 0), stop=(k2 == KC // 2 - 1),
                        perf_mode=DBLROW,
                    )
                dst = QT_sb if j < 4 else KT_sb
                nc.vector.tensor_scalar_add(
                    dst[:, pair, ts(q, 512)], acc[:], bqk_sb[:, j, :])

            def emit_proj_group(i, n, tail=False):
                # c_proj partial: part[128i.., 512n..] = y_local @ W2_local
                acc = psA.tile([128, 512], F32, tag="acc")
                for k in range(FT):
                    nc.tensor.matmul(
                        acc[:], yT_sb[:, k, ts(i, 128)],
                        W2_sb[:, k, ts(n, 512)],
                        start=(k == 0), stop=(k == FT - 1),
                    )
                ot = outp.tile([128, 512], F32, tag="ot")
                if tail:
                    # ACT is exp-idle at the kernel tail; DVE is not
                    nc.scalar.copy(ot[:], acc[:])
                else:
                    nc.vector.tensor_copy(ot[:], acc[:])
                nc.sync.dma_start(part[ts(i, 128), ts(n, 512)], ot[:])

            def emit_attn_chunk(pair, q, filler, every_tile=False):
                # attention for (pair, tq chunk q); pulls filler groups in
                # between to keep the PE busy while ACT runs the exps
                ntk = 4 * q + 4
                pts = []
                for i in range(ntk):
                    off = 128 * (i - 4 * q) if i >= 4 * q else 0
                    sS = psS.tile([128, 2, 512], F32, tag="s")
                    for a in range(2):
                        nc.tensor.matmul(
                            sS[:, a, off:512],
                            KT_sb[64 * a:64 * a + 64, pair, ts(i, 128)],
                            QT_sb[64 * a:64 * a + 64, pair,
                                  512 * q + off: 512 * (q + 1)],
                            start=True, stop=True,
                        )
                    pt = ptp.tile([128, 2, 512], BF16, tag="pt")
                    nc.scalar.activation(
                        pt[:, :, off:512], sS[:, :, off:512], EXP,
                        scale=0.125 / (W1_SCALE * W1_SCALE))
                    if i >= 4 * q:
                        # diagonal 128x128 sub-block: zero where tk > tq
                        for a in range(2):
                            nc.gpsimd.affine_select(
                                out=pt[:, a, off:off + 128],
                                in_=pt[:, a, off:off + 128],
                                compare_op=mybir.AluOpType.is_ge, fill=0.0,
                                base=0, pattern=[[1, 128]],
                                channel_multiplier=-1,
                            )
                    pts.append((pt, off))
                    if every_tile or i % 3 == 2:
                        for f in filler.take():
                            f()
                for a in range(2):
                    h = 2 * pair + a
                    U = psU.tile([65, 512], F32, tag="u")
                    for i, (pt, off) in enumerate(pts):
                        nc.tensor.matmul(
                            U[:, off:512],
                            V_sb[:, i, h, :],
                            pt[:, a, off:512],
                            start=(i == 0), stop=(i == ntk - 1),
                        )
                    # custom-DVE bitwise op requires SBUF input: stage
                    # the denominator row out of PSUM first
                    den = rcp.tile([1, 512], F32, tag="den")
                    nc.vector.tensor_copy(den[:], U[64:65, :])
                    recip = rcp.tile([1, 512], F32, tag="recip")
                    nc.vector.reciprocal_approx_fast(recip[:], den[:])
                    bcs = bcop.tile([64, 512], F32, tag="bcs")
                    nc.gpsimd.partition_broadcast(bcs[:], recip[:])
                    nc.vector.tensor_mul(
                        yT_sb[64 * a:64 * a + 64, pair, ts(q, 512)],
                        U[0:64, :], bcs[:])
                    for f in filler.take():
                        f()

            class Filler:
                """Doles out deferred, tagged PE work groups a few at a time.
                force() emits immediately any queued group a consumer is
                about to depend on (dataflow deps only see prior writes)."""

                def __init__(self, per_slot=1):
                    self.groups = []
                    self.per_slot = per_slot

                def take(self):
                    out, self.groups = (self.groups[:self.per_slot],
                                        self.groups[self.per_slot:])
                    return [f for _, f in out]

                def extend(self, tagged):
                    self.groups.extend(tagged)

                def force(self, pred):
                    keep = []
                    for tag, f in self.groups:
                        if pred(tag):
                            f()
                        else:
                            keep.append((tag, f))
                    self.groups = keep

                def drain(self):
                    for _, f in self.groups:
                        f()
                    self.groups = []

            # startup: only the two projection groups attn(pair0, q0) needs
            emit_qk_group(0, 0, 0)
            emit_qk_group(0, 4, 0)

            filler = Filler()
            for q in range(NQ):
                for pair in range(PAIRS):
                    if q == 0 and pair == 0:
                        # first chunk: V tiles 0-3 feed PV almost at once
                        for i in range(4):
                            emit_v_group(i)
                    # queue work for upcoming consumers, in need order
                    if pair < PAIRS - 1:
                        npair = pair + 1
                        filler.extend([
                            (("qk", npair, q),
                             lambda p_=npair, j_=j_, q_=q: emit_qk_group(p_, j_, q_))
                            for j_ in (npair, 4 + npair)
                        ])
                    elif q < NQ - 1:
                        filler.extend([
                            (("qk", 0, q + 1),
                             lambda j_=j_, q_=q + 1: emit_qk_group(0, j_, q_))
                            for j_ in (0, 4)
                        ])
                    if q < NQ - 1:
                        # V tiles for chunk q+1, spread over pairs 1-2
                        if pair == 1:
                            filler.extend([
                                (("v", i), lambda i=i: emit_v_group(i))
                                for i in (4 * q + 4, 4 * q + 5)
                            ])
                        elif pair == 2:
                            filler.extend([
                                (("v", i), lambda i=i: emit_v_group(i))
                                for i in (4 * q + 6, 4 * q + 7)
                            ])
                    # correctness: everything attn(pair, q) reads must be
                    # emitted before it (QK of this (pair, q); V tiles < ntk)
                    filler.force(lambda tag, p_=pair, q_=q: (
                        tag == ("qk", p_, q_)
                        or (tag[0] == "v" and tag[1] < 4 * q_ + 4)))
                    emit_attn_chunk(pair, q, filler,
                                    every_tile=(q <= 1))
                # yT chunk q complete across all pairs: its c_proj tiles
                # become filler for chunk q+1 (drained at the end for q=3)
                filler.extend([
                    (("proj", q),
                     lambda i=i, n=n, t=(q == NQ - 1):
                     emit_proj_group(i, n, tail=t))
                    for i in range(4 * q, 4 * q + 4)
                    for n in range(2)
                ])
            filler.drain()

    nc.compile()
    return nc


def _get_nc():
    if "nc" not in _CACHE:
        _CACHE["nc"] = _build()
    return _CACHE["nc"]


def _prep_in_maps(x, W_attn, b_attn, W_proj):
    bf = ml_dtypes.bfloat16
    f8 = ml_dtypes.float8_e4m3
    in_maps = []
    gw = {}
    for g in range(NG):
        s = slice(512 * g, 512 * g + 512)
        W1l = np.concatenate(
            [W_attn[:, 0 * C:][:, s], W_attn[:, 1 * C:][:, s]], axis=1
        ) * W1_SCALE
        # j-major pack: [p, j, k, c] so each j-tile is one contiguous DMA
        W1l = np.ascontiguousarray(
            W1l.reshape(KC, 128, 8, 128).transpose(1, 2, 0, 3)
        ).astype(f8)
        Wvl = W_attn[:, 2 * C:][:, s].astype(bf)
        W2l = np.ascontiguousarray(W_proj[s, :]).astype(bf)
        bqkl = (np.concatenate(
            [b_attn[0 * C:][s], b_attn[1 * C:][s]]
        ) * W1_SCALE).astype(np.float32).reshape(8, 128, 1)
        bvl = b_attn[2 * C:][s].astype(np.float32).reshape(1, 512)
        gw[g] = (W1l, Wvl, W2l, bqkl, bvl)
    for b in range(B):
        xTb = np.ascontiguousarray(x[b].T)
        xTl = xTb.astype(bf)
        xT8l = xTb.astype(f8)
        for g in range(NG):
            W1l, Wvl, W2l, bqkl, bvl = gw[g]
            in_maps.append({"xT": xTl, "xT8": xT8l, "W1": W1l, "Wv": Wvl,
                            "W2": W2l, "bqk": bqkl, "bv": bvl})
    return in_maps


LAST_RESULTS = None


def kernel(x, W_attn, b_attn, W_proj, b_proj):
    global LAST_RESULTS
    nc = _get_nc()
    in_maps = _prep_in_maps(np.asarray(x, np.float32),
                            np.asarray(W_attn, np.float32),
                            np.asarray(b_attn, np.float32),
                            np.asarray(W_proj, np.float32))
    res = bass_utils.run_bass_kernel_spmd(nc, in_maps,
                                          core_ids=list(range(N_CORES)))
    LAST_RESULTS = res
    out = np.empty((B, T, C), np.float32)
    bp = np.asarray(b_proj, np.float32)
    for b in range(B):
        out[b] = res.results[2 * b]["part"] + res.results[2 * b + 1]["part"] + bp
    return out


# revision 14
# speedup vs baseline: 1.1731x; 1.1033x over previous
"""Causal self-attention (B=4, T=2048, C=1024, H=16) on 8 Trainium2 cores.

Sharding: 2-way tensor parallel over head groups (8 heads each) x 4-way data
parallel over batch. Each core computes, for its (batch, head-group):
  - Q/K projection in transposed layout (Q^T, K^T = W^T @ x^T), bf16
  - V projection in natural [t, d] layout, bf16, with a ones-column appended
    per head so the PV matmul also produces the softmax denominator
  - causal attention in S^T = K Q^T orientation: exp (no max subtraction --
    logits are bounded ~O(3) for this problem scale), causal mask on diagonal
    128x128 sub-blocks, PV matmul accumulating U^T = [V|1]^T P^T
  - normalization y^T = U^T[:64] * (1/denom) broadcast via K=1 outer product
  - partial c_proj: part = y_local @ W_proj[rows of local heads]
Host sums the two head-group partials per batch and adds b_proj.

Head pairs are packed onto the 128x128 PE array (partitions 0-63 / 64-127)
so the K=64 S^T matmuls run concurrently in distinct row groups, and both
heads' scores share one [128, 2, 512] PSUM tile so a single ACTIVATE(Exp)
covers the pair (halves the per-instruction overhead on the scalar engine).

Schedule is chunk-major (tq chunk outer, head-pair inner) so c_proj tiles of
chunk q run as PE filler during chunk q+1 instead of crowding the kernel
tail, and the startup emits only the two QK projection groups the first
attention chunk needs (the rest arrive as filler while attention runs).
W1 is pre-packed j-major on the host so each projection tile's weights
arrive in one contiguous-run DMA slice, ordered by first use.
"""

import sys

sys.path.insert(0, "/opt/trn_rl_repo")

import numpy as np
import ml_dtypes

import concourse.bass as bass
import concourse.tile as tile
from concourse import mybir, bacc
from concourse import bass_utils
from concourse.bass import ts

# bass_utils imports antenv.axon_hooks when BASS_TRACE is set; the agent
# image's antenv may lack that module, so provide a no-op registry rather
# than crashing (tracing then degrades gracefully).
try:
    import antenv.axon_hooks  # noqa: F401
except ImportError:
    import types as _types
    import antenv as _antenv

    _ah = _types.ModuleType("antenv.axon_hooks")
    _ah._hook = None
    _ah.set_axon_ntff_profile_hook = lambda h, _m=_ah: setattr(_m, "_hook", h)
    _ah.get_axon_ntff_profile_hook = lambda _m=_ah: _m._hook
    sys.modules["antenv.axon_hooks"] = _ah
    _antenv.axon_hooks = _ah

BF16 = mybir.dt.bfloat16
F32 = mybir.dt.float32
FP8 = mybir.dt.float8e4
DBLROW = mybir.MatmulPerfMode.DoubleRow
# Q/K projection runs in fp8e4 DoubleRow (2 contraction tiles per pass).
# W1 is pre-scaled by W1_SCALE on the host so its 0.02-std weights clear the
# e4m3 subnormal threshold; Q^T/K^T come out W1_SCALE x too large, which the
# exp's logit scale divides back out (bqk is pre-scaled to match).
W1_SCALE = 256.0

B, T, C = 4, 2048, 1024
H, D = 16, 64
NG = 2               # head groups (tensor parallel)
HL = H // NG         # 8 local heads
PAIRS = HL // 2      # 4 head pairs (row/partition packing)
KC = C // 128        # 8 contraction tiles for projections
NT = T // 128        # 16 t tiles
NQ = T // 512        # 4 tq chunks
FT = (HL * D) // 128  # 4 feature tiles for c_proj contraction
N_CORES = 8

_CACHE = {}


def _build():
    nc = bacc.Bacc("TRN2", target_bir_lowering=False, debug=False,
                   num_devices=N_CORES)
    xT = nc.dram_tensor("xT", [C, T], BF16, kind="ExternalInput")
    xT8 = nc.dram_tensor("xT8", [C, T], FP8, kind="ExternalInput")
    W1 = nc.dram_tensor("W1", [128, 8, KC, 128], FP8, kind="ExternalInput")
    Wv = nc.dram_tensor("Wv", [C, HL * D], BF16, kind="ExternalInput")
    W2 = nc.dram_tensor("W2", [HL * D, C], BF16, kind="ExternalInput")
    bqk = nc.dram_tensor("bqk", [8, 128, 1], F32, kind="ExternalInput")
    bv = nc.dram_tensor("bv", [1, HL * D], F32, kind="ExternalInput")
    part = nc.dram_tensor("part", [T, C], F32, kind="ExternalOutput")

    EXP = mybir.ActivationFunctionType.Exp

    with tile.TileContext(nc) as tc:
        with (
            tc.tile_pool(name="const", bufs=1) as constp,
            tc.tile_pool(name="xw", bufs=1) as xw,
            tc.tile_pool(name="qkv", bufs=1) as qkv,
            tc.tile_pool(name="ytp", bufs=1) as ytp,
            tc.tile_pool(name="pt", bufs=20) as ptp,
            tc.tile_pool(name="rc", bufs=3) as rcp,
            tc.tile_pool(name="bco", bufs=3) as bcop,
            tc.tile_pool(name="outp", bufs=3) as outp,
            tc.tile_pool(name="psA", bufs=2, space="PSUM") as psA,
            tc.tile_pool(name="psS", bufs=2, space="PSUM") as psS,
            tc.tile_pool(name="psU", bufs=2, space="PSUM") as psU,
        ):
            # ---- constants / weights ----
            ones64 = constp.tile([1, 64], F32, tag="ones64")
            nc.vector.memset(ones64[:], 1.0)
            ones128 = constp.tile([1, 128], F32, tag="ones128")
            nc.vector.memset(ones128[:], 1.0)

            # DMA schedule: sync (HWDGE) carries the startup-critical stream
            # in exact first-use order; W2 rides the scalar ring in parallel.
            bqk_sb = constp.tile([128, 8, 1], F32, tag="bqk")
            nc.sync.dma_start(bqk_sb[:], bqk.rearrange("j p o -> p j o"))
            bv_sb = constp.tile([1, HL * D], F32, tag="bv")
            nc.sync.dma_start(bv_sb[:], bv[:])

            xT8_sb = xw.tile([128, KC, T], FP8, tag="xT8")
            xT8r = xT8.rearrange("(k p) t -> p k t", p=128)
            nc.sync.dma_start(xT8_sb[:, :, ts(0, 512)], xT8r[:, :, ts(0, 512)])
            W1_sb = xw.tile([128, 8, KC, 128], FP8, tag="W1")
            for j in (0, 4):
                nc.sync.dma_start(W1_sb[:, j], W1[:, j])
            xT_sb = xw.tile([128, KC, T], BF16, tag="xT")
            xTr = xT.rearrange("(k p) t -> p k t", p=128)
            nc.sync.dma_start(xT_sb[:, :, ts(0, 512)], xTr[:, :, ts(0, 512)])
            Wv_sb = xw.tile([128, KC, HL * D], BF16, tag="Wv")
            nc.sync.dma_start(Wv_sb[:], Wv.rearrange("(k p) m -> p k m", p=128))
            for j in (1, 5, 2, 6, 3, 7):
                nc.sync.dma_start(W1_sb[:, j], W1[:, j])
            nc.sync.dma_start(xT8_sb[:, :, ts(1, 512)], xT8r[:, :, ts(1, 512)])
            for q in range(1, NQ):
                nc.sync.dma_start(xT_sb[:, :, ts(q, 512)],
                                  xTr[:, :, ts(q, 512)])
                if q < NQ - 1:
                    nc.sync.dma_start(xT8_sb[:, :, ts(q + 1, 512)],
                                      xT8r[:, :, ts(q + 1, 512)])
            W2_sb = xw.tile([128, FT, C], BF16, tag="W2")
            nc.scalar.dma_start(W2_sb[:], W2.rearrange("(k p) m -> p k m", p=128))

            # bv broadcast to all 128 t-rows: [128, 512] f32
            bvb_ps = psU.tile([128, HL * D], F32, tag="u")
            nc.tensor.matmul(bvb_ps[:], ones128[:], bv_sb[:], start=True, stop=True)
            bvb = constp.tile([128, HL * D], F32, tag="bvb")
            nc.vector.tensor_copy(bvb[:], bvb_ps[:])

            V_sb = qkv.tile([128, NT, HL, 65], BF16, tag="V")
            # only the per-head ones-column needs the memset; the V columns
            # are fully overwritten by the projection's bias-add below
            nc.gpsimd.memset(V_sb[:, :, :, 64:65], 1.0)
            QT_sb = qkv.tile([128, PAIRS, T], BF16, tag="QT")
            KT_sb = qkv.tile([128, PAIRS, T], BF16, tag="KT")
            yT_sb = ytp.tile([128, PAIRS, T], BF16, tag="yT")

            def emit_v_group(i):
                # V projection t-tile i: V[t, d] (+bias), ones col per head
                acc = psA.tile([128, 512], F32, tag="acc")
                for k in range(KC):
                    nc.tensor.matmul(
                        acc[:], xT_sb[:, k, ts(i, 128)], Wv_sb[:, k, :],
                        start=(k == 0), stop=(k == KC - 1),
                    )
                # single strided add: psum [128,(8,64)] + bias -> V cols 0..63
                # of each 65-wide head block (col 64 stays the memset 1.0)
                nc.vector.tensor_add(
                    V_sb[:, i, :, 0:64],
                    acc[:].rearrange("p (h c) -> p h c", c=64),
                    bvb[:].rearrange("p (h c) -> p h c", c=64))

            def emit_qk_group(pair, j, q):
                # Q/K projection: one [128, 512] output tile of Q^T or K^T,
                # fp8e4 DoubleRow -- each pass contracts 2 k-tiles (K=256)
                acc = psA.tile([128, 512], F32, tag="acc")
                for k2 in range(KC // 2):
                    nc.tensor.matmul(
                        acc[:], W1_sb[:, j, 2 * k2:2 * k2 + 2, :],
                        xT8_sb[:, 2 * k2:2 * k2 + 2, ts(q, 512)],
                        start=(k2 == 0), stop=(k2 == KC // 2 - 1),
                        perf_mode=DBLROW,
                    )
                dst = QT_sb if j < 4 else KT_sb
                nc.vector.tensor_scalar_add(
                    dst[:, pair, ts(q, 512)], acc[:], bqk_sb[:, j, :])

            def emit_proj_group(i, n, tail=False):
                # c_proj partial: part[128i.., 512n..] = y_local @ W2_local
                acc = psA.tile([128, 512], F32, tag="acc")
                for k in range(FT):
                    nc.tensor.matmul(
                        acc[:], yT_sb[:, k, ts(i, 128)],
                        W2_sb[:, k, ts(n, 512)],
                        start=(k == 0), stop=(k == FT - 1),
                    )
                ot = outp.tile([128, 512], F32, tag="ot")
                if tail:
                    # ACT is exp-idle at the kernel tail; DVE is not
                    nc.scalar.copy(ot[:], acc[:])
                else:
                    nc.vector.tensor_copy(ot[:], acc[:])
                nc.sync.dma_start(part[ts(i, 128), ts(n, 512)], ot[:])

            EXPSCALE = 0.125 / (W1_SCALE * W1_SCALE)

            class Filler:
                """Three-lane queue of deferred emission closures.

                front: next (pair, q)'s S^T+exp tile closures -- served first
                       so the scalar engine's exp stream never dries up while
                       the PE runs the current pair's PV chains.
                mid:   QK / V projection groups (always runnable).
                back:  c_proj groups; their yT inputs trail the norm chain,
                       so serving them last avoids PE head-of-line stalls.
                """

                def __init__(self):
                    self.front = []
                    self.mid = []
                    self.back = []

                def take(self, n=1, front=True):
                    # front tiles allocate pt buffers, so they are never
                    # served unless the caller's budget allows (front=True)
                    out = []
                    while n > 0:
                        if front and self.front:
                            out.append(self.front.pop(0)[1])
                        elif self.mid:
                            out.append(self.mid.pop(0)[1])
                        elif self.back:
                            out.append(self.back.pop(0)[1])
                        else:
                            break
                        n -= 1
                    return out

                def force_mid(self, pred):
                    keep = []
                    for tag, f in self.mid:
                        if pred(tag):
                            f()
                        else:
                            keep.append((tag, f))
                    self.mid = keep

                def drain(self):
                    for lane in (self.front, self.mid, self.back):
                        for _, f in lane:
                            f()
                    self.front, self.mid, self.back = [], [], []

            def make_tile_closure(pair, q, i, pts):
                # one S^T key tile + its exp (+ causal mask on the diagonal)
                def f():
                    off = 128 * (i - 4 * q) if i >= 4 * q else 0
                    sS = psS.tile([128, 2, 512], F32, tag="s")
                    for a in range(2):
                        nc.tensor.matmul(
                            sS[:, a, off:512],
                            KT_sb[64 * a:64 * a + 64, pair, ts(i, 128)],
                            QT_sb[64 * a:64 * a + 64, pair,
                                  512 * q + off: 512 * (q + 1)],
                            start=True, stop=True,
                        )
                    pt = ptp.tile([128, 2, 512], BF16, tag="pt")
                    nc.scalar.activation(
                        pt[:, :, off:512], sS[:, :, off:512], EXP,
                        scale=EXPSCALE)
                    if i >= 4 * q:
                        # diagonal 128x128 sub-block: zero where tk > tq
                        for a in range(2):
                            nc.gpsimd.affine_select(
                                out=pt[:, a, off:off + 128],
                                in_=pt[:, a, off:off + 128],
                                compare_op=mybir.AluOpType.is_ge, fill=0.0,
                                base=0, pattern=[[1, 128]],
                                channel_multiplier=-1,
                            )
                    pts.append((pt, off))
                return f

            tiles_pending = {}   # (pair, q) -> list of un-emitted closures
            pts_map = {}         # (pair, q) -> accumulated (pt, off) list
            prepared = set()

            def prepare_tiles(pair, q):
                assert (pair, q) not in prepared
                prepared.add((pair, q))
                pts = []
                pts_map[(pair, q)] = pts
                tiles_pending[(pair, q)] = [
                    make_tile_closure(pair, q, i, pts)
                    for i in range(4 * q + 4)
                ]

            def emit_U(pair, q, filler, front_budget):
                # PV accumulation + normalization for both heads, pulling
                # filler between sub-chains; front pulls capped so the pt
                # pool is not overcommitted
                ntk = 4 * q + 4
                pts = pts_map.pop((pair, q))
                budget = [front_budget]

                def pull(n):
                    use_front = budget[0] > 0
                    got = filler.take(n, front=use_front)
                    budget[0] -= n
                    for f in got:
                        f()

                for a in range(2):
                    h = 2 * pair + a
                    U = psU.tile([65, 512], F32, tag="u")
                    for i, (pt, off) in enumerate(pts):
                        nc.tensor.matmul(
                            U[:, off:512],
                            V_sb[:, i, h, :],
                            pt[:, a, off:512],
                            start=(i == 0), stop=(i == ntk - 1),
                        )
                        if i % 4 == 3 and i < ntk - 1:
                            pull(1)
                    # custom-DVE bitwise op requires SBUF input: stage
                    # the denominator row out of PSUM first
                    den = rcp.tile([1, 512], F32, tag="den")
                    nc.vector.tensor_copy(den[:], U[64:65, :])
                    recip = rcp.tile([1, 512], F32, tag="recip")
                    nc.vector.reciprocal_approx_fast(recip[:], den[:])
                    bcs = bcop.tile([64, 512], F32, tag="bcs")
                    nc.gpsimd.partition_broadcast(bcs[:], recip[:])
                    nc.vector.tensor_mul(
                        yT_sb[64 * a:64 * a + 64, pair, ts(q, 512)],
                        U[0:64, :], bcs[:])
                    pull(1)

            # startup: only the two projection groups attn(pair0, q0) needs
            emit_qk_group(0, 0, 0)
            emit_qk_group(0, 4, 0)
            for i in range(4):
                emit_v_group(i)

            filler = Filler()
            prepare_tiles(0, 0)
            for q in range(NQ):
                for pair in range(PAIRS):
                    # queue projection work for upcoming consumers
                    if pair < PAIRS - 1:
                        npair = pair + 1
                        filler.mid.extend([
                            (("qk", npair, q),
                             lambda p_=npair, j_=j_, q_=q: emit_qk_group(p_, j_, q_))
                            for j_ in (npair, 4 + npair)
                        ])
                    elif q < NQ - 1:
                        filler.mid.extend([
                            (("qk", 0, q + 1),
                             lambda j_=j_, q_=q + 1: emit_qk_group(0, j_, q_))
                            for j_ in (0, 4)
                        ])
                    if q < NQ - 1:
                        if pair == 1:
                            filler.mid.extend([
                                (("v", i), lambda i=i: emit_v_group(i))
                                for i in (4 * q + 4, 4 * q + 5)
                            ])
                        elif pair == 2:
                            filler.mid.extend([
                                (("v", i), lambda i=i: emit_v_group(i))
                                for i in (4 * q + 6, 4 * q + 7)
                            ])
                    # emit this pair's remaining S^T tiles (some were already
                    # pulled as front-lane filler during the previous U phase),
                    # with mid/back pulls interleaved to keep the PE dense
                    rem = tiles_pending.pop((pair, q))
                    for k, f in enumerate(rem):
                        f()
                        if k % 3 == 2:
                            for g in filler.take(1, front=False):
                                g()
                    # stage the NEXT attention tiles into the front lane
                    nxt = (pair + 1, q) if pair < PAIRS - 1 else \
                          ((0, q + 1) if q < NQ - 1 else None)
                    if nxt is not None:
                        np_, nq_ = nxt
                        # its QK projection must be fully emitted first
                        filler.force_mid(lambda tag: tag == ("qk", np_, nq_))
                        if nq_ != q:
                            filler.force_mid(lambda tag: (
                                tag[0] == "v" and tag[1] < 4 * nq_ + 4))
                        prepare_tiles(np_, nq_)
                        pend = tiles_pending[(np_, nq_)]
                        filler.front.extend([(("st", np_, nq_, i), pf)
                                             for i, pf in enumerate(pend)])
                        # front lane aliases tiles_pending: consume jointly
                        tiles_pending[(np_, nq_)] = []

                        def reclaim(np__=np_, nq__=nq_):
                            # anything still in front belongs to (np_, nq_)
                            left = [f for t, f in filler.front
                                    if t[:3] == ("st", np__, nq__)]
                            filler.front = [e for e in filler.front
                                            if e[0][:3] != ("st", np__, nq__)]
                            return left
                        tiles_reclaim = reclaim
                    # PV + normalization; cap front pulls by free pt buffers
                    emit_U(pair, q, filler,
                           front_budget=max(0, 20 - (4 * q + 4) - 1))
                    if nxt is not None:
                        tiles_pending[nxt[0], nxt[1]] = tiles_reclaim()
                # yT chunk q complete across all pairs
                filler.back.extend([
                    (("proj", q),
                     lambda i=i, n=n, t=(q == NQ - 1):
                     emit_proj_group(i, n, tail=t))
                    for i in range(4 * q, 4 * q + 4)
                    for n in range(2)
                ])
            filler.drain()

    nc.compile()
    return nc


def _get_nc():
    if "nc" not in _CACHE:
        _CACHE["nc"] = _build()
    return _CACHE["nc"]


def _prep_in_maps(x, W_attn, b_attn, W_proj):
    bf = ml_dtypes.bfloat16
    f8 = ml_dtypes.float8_e4m3
    in_maps = []
    gw = {}
    for g in range(NG):
        s = slice(512 * g, 512 * g + 512)
        W1l = np.concatenate(
            [W_attn[:, 0 * C:][:, s], W_attn[:, 1 * C:][:, s]], axis=1
        ) * W1_SCALE
        # j-major pack: [p, j, k, c] so each j-tile is one contiguous DMA
        W1l = np.ascontiguousarray(
            W1l.reshape(KC, 128, 8, 128).transpose(1, 2, 0, 3)
        ).astype(f8)
        Wvl = W_attn[:, 2 * C:][:, s].astype(bf)
        W2l = np.ascontiguousarray(W_proj[s, :]).astype(bf)
        bqkl = (np.concatenate(
            [b_attn[0 * C:][s], b_attn[1 * C:][s]]
        ) * W1_SCALE).astype(np.float32).reshape(8, 128, 1)
        bvl = b_attn[2 * C:][s].astype(np.float32).reshape(1, 512)
        gw[g] = (W1l, Wvl, W2l, bqkl, bvl)
    for b in range(B):
        xTb = np.ascontiguousarray(x[b].T)
        xTl = xTb.astype(bf)
        xT8l = xTb.astype(f8)
        for g in range(NG):
            W1l, Wvl, W2l, bqkl, bvl = gw[g]
            in_maps.append({"xT": xTl, "xT8": xT8l, "W1": W1l, "Wv": Wvl,
                            "W2": W2l, "bqk": bqkl, "bv": bvl})
    return in_maps


LAST_RESULTS = None


def kernel(x, W_attn, b_attn, W_proj, b_proj):
    global LAST_RESULTS
    nc = _get_nc()
    in_maps = _prep_in_maps(np.asarray(x, np.float32),
                            np.asarray(W_attn, np.float32),
                            np.asarray(b_attn, np.float32),
                            np.asarray(W_proj, np.float32))
    res = bass_utils.run_bass_kernel_spmd(nc, in_maps,
                                          core_ids=list(range(N_CORES)))
    LAST_RESULTS = res
    out = np.empty((B, T, C), np.float32)
    bp = np.asarray(b_proj, np.float32)
    for b in range(B):
        out[b] = res.results[2 * b]["part"] + res.results[2 * b + 1]["part"] + bp
    return out


# revision 20
# speedup vs baseline: 1.2397x; 1.0568x over previous
"""Causal self-attention (B=4, T=2048, C=1024, H=16) on 8 Trainium2 cores.

Sharding: 2-way tensor parallel over head groups (8 heads each) x 4-way data
parallel over batch. Each core computes, for its (batch, head-group):
  - Q/K projection in transposed layout (Q^T, K^T = W^T @ x^T), bf16
  - V projection in natural [t, d] layout, bf16, with a ones-column appended
    per head so the PV matmul also produces the softmax denominator
  - causal attention in S^T = K Q^T orientation: exp (no max subtraction --
    logits are bounded ~O(3) for this problem scale), causal mask on diagonal
    128x128 sub-blocks, PV matmul accumulating U^T = [V|1]^T P^T
  - normalization y^T = U^T[:64] * (1/denom) broadcast via K=1 outer product
  - partial c_proj: part = y_local @ W_proj[rows of local heads]
Host sums the two head-group partials per batch and adds b_proj.

Head pairs are packed onto the 128x128 PE array (partitions 0-63 / 64-127)
so the K=64 S^T matmuls run concurrently in distinct row groups, and both
heads' scores share one [128, 2, 512] PSUM tile so a single ACTIVATE(Exp)
covers the pair (halves the per-instruction overhead on the scalar engine).

Schedule is chunk-major (tq chunk outer, head-pair inner) so c_proj tiles of
chunk q run as PE filler during chunk q+1 instead of crowding the kernel
tail, and the startup emits only the two QK projection groups the first
attention chunk needs (the rest arrive as filler while attention runs).
W1 is pre-packed j-major on the host so each projection tile's weights
arrive in one contiguous-run DMA slice, ordered by first use.
"""

import sys

sys.path.insert(0, "/opt/trn_rl_repo")

import numpy as np
import ml_dtypes

import concourse.bass as bass
import concourse.tile as tile
from concourse import mybir, bacc
from concourse import bass_utils
from concourse.bass import ts

# bass_utils imports antenv.axon_hooks when BASS_TRACE is set; the agent
# image's antenv may lack that module, so provide a no-op registry rather
# than crashing (tracing then degrades gracefully).
try:
    import antenv.axon_hooks  # noqa: F401
except ImportError:
    import types as _types
    import antenv as _antenv

    _ah = _types.ModuleType("antenv.axon_hooks")
    _ah._hook = None
    _ah.set_axon_ntff_profile_hook = lambda h, _m=_ah: setattr(_m, "_hook", h)
    _ah.get_axon_ntff_profile_hook = lambda _m=_ah: _m._hook
    sys.modules["antenv.axon_hooks"] = _ah
    _antenv.axon_hooks = _ah

BF16 = mybir.dt.bfloat16
F32 = mybir.dt.float32
FP8 = mybir.dt.float8e4
DBLROW = mybir.MatmulPerfMode.DoubleRow
# Q/K projection runs in fp8e4 DoubleRow (2 contraction tiles per pass).
# W1 is pre-scaled by W1_SCALE on the host so its 0.02-std weights clear the
# e4m3 subnormal threshold; Q^T/K^T come out W1_SCALE x too large, which the
# exp's logit scale divides back out (bqk is pre-scaled to match).
W1_SCALE = 256.0

B, T, C = 4, 2048, 1024
H, D = 16, 64
NG = 2               # head groups (tensor parallel)
HL = H // NG         # 8 local heads
PAIRS = HL // 2      # 4 head pairs (row/partition packing)
KC = C // 128        # 8 contraction tiles for projections
NT = T // 128        # 16 t tiles
NQ = T // 512        # 4 tq chunks
FT = (HL * D) // 128  # 4 feature tiles for c_proj contraction
N_CORES = 8

_CACHE = {}


def _build():
    nc = bacc.Bacc("TRN2", target_bir_lowering=False, debug=False,
                   num_devices=N_CORES)
    xT = nc.dram_tensor("xT", [C, T], BF16, kind="ExternalInput")
    xT8 = nc.dram_tensor("xT8", [C, T], FP8, kind="ExternalInput")
    W1 = nc.dram_tensor("W1", [128, 8, KC, 128], FP8, kind="ExternalInput")
    Wv = nc.dram_tensor("Wv", [C, HL * D], BF16, kind="ExternalInput")
    W2 = nc.dram_tensor("W2", [HL * D, C], BF16, kind="ExternalInput")
    bqk = nc.dram_tensor("bqk", [128, 8, 1], F32, kind="ExternalInput")
    bv = nc.dram_tensor("bv", [1, HL * D], F32, kind="ExternalInput")
    part = nc.dram_tensor("part", [T, C], BF16, kind="ExternalOutput")

    EXP = mybir.ActivationFunctionType.Exp

    with tile.TileContext(nc) as tc:
        with (
            tc.tile_pool(name="const", bufs=1) as constp,
            tc.tile_pool(name="xw", bufs=1) as xw,
            tc.tile_pool(name="qkv", bufs=1) as qkv,
            tc.tile_pool(name="ytp", bufs=1) as ytp,
            tc.tile_pool(name="pt", bufs=20) as ptp,
            tc.tile_pool(name="rc", bufs=3) as rcp,
            tc.tile_pool(name="bco", bufs=3) as bcop,
            tc.tile_pool(name="outp", bufs=3) as outp,
            tc.tile_pool(name="psA", bufs=2, space="PSUM") as psA,
            tc.tile_pool(name="psS", bufs=2, space="PSUM") as psS,
            tc.tile_pool(name="psU", bufs=2, space="PSUM") as psU,
        ):
            # ---- constants / weights ----
            ones64 = constp.tile([1, 64], F32, tag="ones64")
            nc.vector.memset(ones64[:], 1.0)
            ones128 = constp.tile([1, 128], F32, tag="ones128")
            nc.vector.memset(ones128[:], 1.0)

            # DMA schedule: sync (HWDGE) carries the startup-critical stream
            # in exact first-use order. Remainder transfers are fused (one
            # long run per DRAM row) to stay byte-bound, not descriptor-bound.
            bqk_sb = constp.tile([128, 8, 1], F32, tag="bqk")
            nc.sync.dma_start(bqk_sb[:], bqk[:])
            bv_sb = constp.tile([1, HL * D], F32, tag="bv")
            nc.sync.dma_start(bv_sb[:], bv[:])

            xT8_sb = xw.tile([128, KC, T], FP8, tag="xT8")
            xT8r = xT8.rearrange("(k p) t -> p k t", p=128)
            nc.sync.dma_start(xT8_sb[:, :, ts(0, 512)], xT8r[:, :, ts(0, 512)])
            W1_sb = xw.tile([128, 8, KC, 128], FP8, tag="W1")
            for j in (0, 4):
                nc.sync.dma_start(W1_sb[:, j], W1[:, j])
            xT_sb = xw.tile([128, KC, T], BF16, tag="xT")
            xTr = xT.rearrange("(k p) t -> p k t", p=128)
            nc.sync.dma_start(xT_sb[:, :, ts(0, 512)], xTr[:, :, ts(0, 512)])
            Wv_sb = xw.tile([128, KC, HL * D], BF16, tag="Wv")
            nc.sync.dma_start(Wv_sb[:], Wv.rearrange("(k p) m -> p k m", p=128))
            for j in (1, 5, 2, 6, 3, 7):
                nc.sync.dma_start(W1_sb[:, j], W1[:, j])
            nc.sync.dma_start(xT_sb[:, :, ts(1, 512)], xTr[:, :, ts(1, 512)])
            nc.sync.dma_start(xT8_sb[:, :, 512:T], xT8r[:, :, 512:T])
            nc.sync.dma_start(xT_sb[:, :, 1024:T], xTr[:, :, 1024:T])
            W2_sb = xw.tile([128, FT, C], BF16, tag="W2")
            nc.sync.dma_start(W2_sb[:], W2.rearrange("(k p) m -> p k m", p=128))

            # bv broadcast to all 128 t-rows: [128, 512] f32
            bvb_ps = psU.tile([128, HL * D], F32, tag="u")
            nc.tensor.matmul(bvb_ps[:], ones128[:], bv_sb[:], start=True, stop=True)
            bvb = constp.tile([128, HL * D], F32, tag="bvb")
            nc.vector.tensor_copy(bvb[:], bvb_ps[:])

            V_sb = qkv.tile([128, NT, HL, 65], BF16, tag="V")
            # only the per-head ones-column needs the memset; the V columns
            # are fully overwritten by the projection's bias-add below
            nc.gpsimd.memset(V_sb[:, :, :, 64:65], 1.0)

            # warm both gpsimd custom-op libraries now, while the engines sit
            # out the input DMA: the first partition_broadcast otherwise pays
            # a ~7us library load mid-kernel, serializing every U-normalize
            # of chunk 0 behind it
            warm = constp.tile([128, 4], F32, tag="warm")
            nc.vector.memset(warm[:], 0.0)
            nc.gpsimd.affine_select(
                out=warm[:, 0:4], in_=warm[:, 0:4],
                compare_op=mybir.AluOpType.is_ge, fill=0.0,
                base=0, pattern=[[1, 4]], channel_multiplier=-1)
            nc.gpsimd.partition_broadcast(warm[:, 0:1], warm[0:1, 0:1])
            QT_sb = qkv.tile([128, PAIRS, T], BF16, tag="QT")
            KT_sb = qkv.tile([128, PAIRS, T], BF16, tag="KT")
            yT_sb = ytp.tile([128, PAIRS, T], BF16, tag="yT")

            def emit_v_group(i):
                # V projection t-tile i: V[t, d] (+bias), ones col per head
                acc = psA.tile([128, 512], F32, tag="acc")
                for k in range(KC):
                    nc.tensor.matmul(
                        acc[:], xT_sb[:, k, ts(i, 128)], Wv_sb[:, k, :],
                        start=(k == 0), stop=(k == KC - 1),
                    )
                # single strided add: psum [128,(8,64)] + bias -> V cols 0..63
                # of each 65-wide head block (col 64 stays the memset 1.0)
                nc.vector.tensor_add(
                    V_sb[:, i, :, 0:64],
                    acc[:].rearrange("p (h c) -> p h c", c=64),
                    bvb[:].rearrange("p (h c) -> p h c", c=64))

            def emit_qk_group(pair, j, q):
                # Q/K projection: one [128, 512] output tile of Q^T or K^T,
                # fp8e4 DoubleRow -- each pass contracts 2 k-tiles (K=256)
                acc = psA.tile([128, 512], F32, tag="acc")
                for k2 in range(KC // 2):
                    nc.tensor.matmul(
                        acc[:], W1_sb[:, j, 2 * k2:2 * k2 + 2, :],
                        xT8_sb[:, 2 * k2:2 * k2 + 2, ts(q, 512)],
                        start=(k2 == 0), stop=(k2 == KC // 2 - 1),
                        perf_mode=DBLROW,
                    )
                dst = QT_sb if j < 4 else KT_sb
                nc.vector.tensor_scalar_add(
                    dst[:, pair, ts(q, 512)], acc[:], bqk_sb[:, j, :])

            def emit_proj_group(i, n, tail=False):
                # c_proj partial: part[128i.., 512n..] = y_local @ W2_local
                acc = psA.tile([128, 512], F32, tag="acc")
                for k in range(FT):
                    nc.tensor.matmul(
                        acc[:], yT_sb[:, k, ts(i, 128)],
                        W2_sb[:, k, ts(n, 512)],
                        start=(k == 0), stop=(k == FT - 1),
                    )
                ot = outp.tile([128, 512], BF16, tag="ot")
                if tail:
                    # ACT is exp-idle at the kernel tail; DVE is not
                    nc.scalar.copy(ot[:], acc[:])
                else:
                    nc.vector.tensor_copy(ot[:], acc[:])
                nc.sync.dma_start(part[ts(i, 128), ts(n, 512)], ot[:])

            EXPSCALE = 0.125 / (W1_SCALE * W1_SCALE)

            class Filler:
                """Three-lane queue of deferred emission closures.

                front: next (pair, q)'s S^T+exp tile closures -- served first
                       so the scalar engine's exp stream never dries up while
                       the PE runs the current pair's PV chains.
                mid:   QK / V projection groups (always runnable).
                back:  c_proj groups; their yT inputs trail the norm chain,
                       so serving them last avoids PE head-of-line stalls.
                """

                def __init__(self):
                    self.front = []
                    self.mid = []
                    self.back = []

                def take(self, n=1, front=True):
                    # front tiles allocate pt buffers, so they are never
                    # served unless the caller's budget allows (front=True)
                    out = []
                    while n > 0:
                        if front and self.front:
                            out.append(self.front.pop(0)[1])
                        elif self.mid:
                            out.append(self.mid.pop(0)[1])
                        elif self.back:
                            out.append(self.back.pop(0)[1])
                        else:
                            break
                        n -= 1
                    return out

                def force_mid(self, pred):
                    keep = []
                    for tag, f in self.mid:
                        if pred(tag):
                            f()
                        else:
                            keep.append((tag, f))
                    self.mid = keep

                def drain(self):
                    for lane in (self.front, self.mid, self.back):
                        for _, f in lane:
                            f()
                    self.front, self.mid, self.back = [], [], []

            def make_tile_closure(pair, q, i, pts):
                # one S^T key tile + its exp (+ causal mask on the diagonal)
                def f():
                    off = 128 * (i - 4 * q) if i >= 4 * q else 0
                    sS = psS.tile([128, 2, 512], F32, tag="s")
                    for a in range(2):
                        nc.tensor.matmul(
                            sS[:, a, off:512],
                            KT_sb[64 * a:64 * a + 64, pair, ts(i, 128)],
                            QT_sb[64 * a:64 * a + 64, pair,
                                  512 * q + off: 512 * (q + 1)],
                            start=True, stop=True,
                        )
                    pt = ptp.tile([128, 2, 512], BF16, tag="pt")
                    nc.scalar.activation(
                        pt[:, :, off:512], sS[:, :, off:512], EXP,
                        scale=EXPSCALE)
                    if i >= 4 * q:
                        # diagonal 128x128 sub-block: zero where tk > tq
                        for a in range(2):
                            nc.gpsimd.affine_select(
                                out=pt[:, a, off:off + 128],
                                in_=pt[:, a, off:off + 128],
                                compare_op=mybir.AluOpType.is_ge, fill=0.0,
                                base=0, pattern=[[1, 128]],
                                channel_multiplier=-1,
                            )
                    pts.append((pt, off))
                return f

            tiles_pending = {}   # (pair, q) -> list of un-emitted closures
            pts_map = {}         # (pair, q) -> accumulated (pt, off) list
            prepared = set()

            def prepare_tiles(pair, q):
                assert (pair, q) not in prepared
                prepared.add((pair, q))
                pts = []
                pts_map[(pair, q)] = pts
                tiles_pending[(pair, q)] = [
                    make_tile_closure(pair, q, i, pts)
                    for i in range(4 * q + 4)
                ]

            def emit_U(pair, q, filler, front_budget):
                # PV accumulation + normalization for both heads, pulling
                # filler between sub-chains; front pulls capped so the pt
                # pool is not overcommitted
                ntk = 4 * q + 4
                pts = pts_map.pop((pair, q))
                budget = [front_budget]

                def pull(n):
                    use_front = budget[0] > 0
                    got = filler.take(n, front=use_front)
                    budget[0] -= n
                    for f in got:
                        f()

                for a in range(2):
                    h = 2 * pair + a
                    U = psU.tile([65, 512], F32, tag="u")
                    for i, (pt, off) in enumerate(pts):
                        nc.tensor.matmul(
                            U[:, off:512],
                            V_sb[:, i, h, :],
                            pt[:, a, off:512],
                            start=(i == 0), stop=(i == ntk - 1),
                        )
                        if i % 4 == 3 and i < ntk - 1:
                            pull(1)
                    # custom-DVE bitwise op requires SBUF input: stage
                    # the denominator row out of PSUM first
                    den = rcp.tile([1, 512], F32, tag="den")
                    nc.vector.tensor_copy(den[:], U[64:65, :])
                    recip = rcp.tile([1, 512], F32, tag="recip")
                    nc.vector.reciprocal_approx_fast(recip[:], den[:])
                    bcs = bcop.tile([64, 512], F32, tag="bcs")
                    nc.gpsimd.partition_broadcast(bcs[:], recip[:])
                    nc.vector.tensor_mul(
                        yT_sb[64 * a:64 * a + 64, pair, ts(q, 512)],
                        U[0:64, :], bcs[:])
                    pull(1)

            # startup: only the two projection groups attn(pair0, q0) needs
            emit_qk_group(0, 0, 0)
            emit_qk_group(0, 4, 0)
            for i in range(4):
                emit_v_group(i)

            filler = Filler()
            prepare_tiles(0, 0)
            for q in range(NQ):
                for pair in range(PAIRS):
                    # queue projection work for upcoming consumers
                    if pair < PAIRS - 1:
                        npair = pair + 1
                        filler.mid.extend([
                            (("qk", npair, q),
                             lambda p_=npair, j_=j_, q_=q: emit_qk_group(p_, j_, q_))
                            for j_ in (npair, 4 + npair)
                        ])
                    elif q < NQ - 1:
                        filler.mid.extend([
                            (("qk", 0, q + 1),
                             lambda j_=j_, q_=q + 1: emit_qk_group(0, j_, q_))
                            for j_ in (0, 4)
                        ])
                    if q < NQ - 1:
                        if pair == 1:
                            filler.mid.extend([
                                (("v", i), lambda i=i: emit_v_group(i))
                                for i in (4 * q + 4, 4 * q + 5)
                            ])
                        elif pair == 2:
                            filler.mid.extend([
                                (("v", i), lambda i=i: emit_v_group(i))
                                for i in (4 * q + 6, 4 * q + 7)
                            ])
                    # emit this pair's remaining S^T tiles (some were already
                    # pulled as front-lane filler during the previous U phase),
                    # with mid/back pulls interleaved to keep the PE dense
                    rem = tiles_pending.pop((pair, q))
                    for k, f in enumerate(rem):
                        f()
                        if k % 3 == 2:
                            for g in filler.take(1, front=False):
                                g()
                    # stage the NEXT attention tiles into the front lane
                    nxt = (pair + 1, q) if pair < PAIRS - 1 else \
                          ((0, q + 1) if q < NQ - 1 else None)
                    if nxt is not None:
                        np_, nq_ = nxt
                        # its QK projection must be fully emitted first
                        filler.force_mid(lambda tag: tag == ("qk", np_, nq_))
                        if nq_ != q:
                            filler.force_mid(lambda tag: (
                                tag[0] == "v" and tag[1] < 4 * nq_ + 4))
                        prepare_tiles(np_, nq_)
                        pend = tiles_pending[(np_, nq_)]
                        filler.front.extend([(("st", np_, nq_, i), pf)
                                             for i, pf in enumerate(pend)])
                        # front lane aliases tiles_pending: consume jointly
                        tiles_pending[(np_, nq_)] = []

                        def reclaim(np__=np_, nq__=nq_):
                            # anything still in front belongs to (np_, nq_)
                            left = [f for t, f in filler.front
                                    if t[:3] == ("st", np__, nq__)]
                            filler.front = [e for e in filler.front
                                            if e[0][:3] != ("st", np__, nq__)]
                            return left
                        tiles_reclaim = reclaim
                    # PV + normalization; cap front pulls by free pt buffers
                    emit_U(pair, q, filler,
                           front_budget=max(0, 20 - (4 * q + 4) - 1))
                    if nxt is not None:
                        tiles_pending[nxt[0], nxt[1]] = tiles_reclaim()
                # yT chunk q complete across all pairs
                filler.back.extend([
                    (("proj", q),
                     lambda i=i, n=n, t=(q == NQ - 1):
                     emit_proj_group(i, n, tail=t))
                    for i in range(4 * q, 4 * q + 4)
                    for n in range(2)
                ])
            filler.drain()

    nc.compile()
    return nc


def _get_nc():
    if "nc" not in _CACHE:
        _CACHE["nc"] = _build()
    return _CACHE["nc"]


def _prep_in_maps(x, W_attn, b_attn, W_proj):
    bf = ml_dtypes.bfloat16
    f8 = ml_dtypes.float8_e4m3
    in_maps = []
    gw = {}
    for g in range(NG):
        s = slice(512 * g, 512 * g + 512)
        W1l = np.concatenate(
            [W_attn[:, 0 * C:][:, s], W_attn[:, 1 * C:][:, s]], axis=1
        ) * W1_SCALE
        # j-major pack: [p, j, k, c] so each j-tile is one contiguous DMA
        W1l = np.ascontiguousarray(
            W1l.reshape(KC, 128, 8, 128).transpose(1, 2, 0, 3)
        ).astype(f8)
        Wvl = W_attn[:, 2 * C:][:, s].astype(bf)
        W2l = np.ascontiguousarray(W_proj[s, :]).astype(bf)
        bqkl = np.ascontiguousarray((np.concatenate(
            [b_attn[0 * C:][s], b_attn[1 * C:][s]]
        ) * W1_SCALE).astype(np.float32).reshape(8, 128).T).reshape(128, 8, 1)
        bvl = b_attn[2 * C:][s].astype(np.float32).reshape(1, 512)
        gw[g] = (W1l, Wvl, W2l, bqkl, bvl)
    for b in range(B):
        xTb = np.ascontiguousarray(x[b].T)
        xTl = xTb.astype(bf)
        xT8l = xTb.astype(f8)
        for g in range(NG):
            W1l, Wvl, W2l, bqkl, bvl = gw[g]
            in_maps.append({"xT": xTl, "xT8": xT8l, "W1": W1l, "Wv": Wvl,
                            "W2": W2l, "bqk": bqkl, "bv": bvl})
    return in_maps


LAST_RESULTS = None


def kernel(x, W_attn, b_attn, W_proj, b_proj):
    global LAST_RESULTS
    nc = _get_nc()
    in_maps = _prep_in_maps(np.asarray(x, np.float32),
                            np.asarray(W_attn, np.float32),
                            np.asarray(b_attn, np.float32),
                            np.asarray(W_proj, np.float32))
    res = bass_utils.run_bass_kernel_spmd(nc, in_maps,
                                          core_ids=list(range(N_CORES)))
    LAST_RESULTS = res
    out = np.empty((B, T, C), np.float32)
    bp = np.asarray(b_proj, np.float32)
    for b in range(B):
        out[b] = (res.results[2 * b]["part"].astype(np.float32)
                  + res.results[2 * b + 1]["part"].astype(np.float32) + bp)
    return out
